# revision 1
# baseline (speedup 1.0000x reference)
"""Trainium2 Bass kernel for nn_ClassifierModel (nms_detection).

Computation (reference):
    h    = relu(features @ conv_w + conv_b)        # (B,H,W,C)@(C,D) -> (B,H,W,D)
    flat = h.reshape(B, F)                         # F = H*W*D = 401408
    cls  = flat @ cls_w + cls_b                    # (B, 64)
    bbox = flat @ bbox_w + bbox_b                  # (B, 128)
    <tiny postprocessing with roi -> (B, P, 5)>

Sharding: the flatten (contraction) dim F is split across the 8 cores by
slicing H into 8 chunks of 28 rows. Each core computes its conv slice and a
partial (B, 192) product against its slice of [cls_w | bbox_w]; the host sums
the 8 partials and runs the tiny postprocessing. This reads each dense-weight
element exactly once across the machine (the weights dominate HBM traffic).

Per-core device layout (matmul operands bf16, accumulation fp32 in PSUM):
    featT  (4,128,NB) : features slice, transposed to (c, pix*B+b) columns, bf16
    convw  (4,128,256): conv_w k-tiles (c on partitions), bf16
    convb  (2,128,1)  : conv_b halves (d on partitions), fp32
    wmat   (128,NT*192): [cls|bbox] rows f-tiled in q-major consumption order
    out    (16,192)   : partial fp32 [cls|bbox] sums for this core's f range

Stage 1 produces h^T with d on partitions and (pix, b) on the free axis; a
128-partition f-tile of flat^T is then exactly hT[q][:, pix*16:(pix+1)*16],
so stage 2 needs no transposes at all. Stage 2 consumes f-tiles in q-major
order (all q=0 tiles, then q=1) so it can start as soon as hT[0] exists; the
host lays wmat out in the same order. The W stream is chunked with small tail
chunks so the final chunk's matmul tail is short.
"""

import numpy as np

B = 16
H, W, C = 224, 7, 512
D = 256
P = 32
NCORES = 8
HSH = H // NCORES          # 28 rows of H per core
PIX = HSH * W              # 196 pixels per core per batch
FLOC = PIX * D             # 50176 contraction elements per core
NB = PIX * B               # 3136 stage-1 moving columns
NT = FLOC // 128           # 392 f-tiles per core
NQ = NT // 2               # 196 f-tiles per d-half
NTILE = 448                # stage-1 moving tile (3136 = 7*448)
CHUNKS = [42] * 8 + [28, 14, 7, 4, 3]   # W-stream chunks (sum = 392)
STRIDE = 16.0

_STATE = {}


def _build_module(reps=1):
    import concourse.mybir as mybir
    import concourse.tile as tile
    from concourse import bacc

    f32 = mybir.dt.float32
    bf16 = mybir.dt.bfloat16
    nc = bacc.Bacc("TRN2", target_bir_lowering=False, debug=False)

    featT = nc.dram_tensor("featT", [4, 128, NB], bf16, kind="ExternalInput")
    convw = nc.dram_tensor("convw", [4, 128, D], bf16, kind="ExternalInput")
    convb = nc.dram_tensor("convb", [2, 128, 1], f32, kind="ExternalInput")
    wmat = nc.dram_tensor("wmat", [128, NT * 192], bf16, kind="ExternalInput")
    if reps == 1:
        out = nc.dram_tensor("out", [16, 192], f32, kind="ExternalOutput")
    else:
        out = nc.dram_tensor("out", [reps, 16, 192], f32, kind="ExternalOutput")

    NTI = NB // NTILE  # 7 stage-1 n-tiles

    with tile.TileContext(nc) as tc:
        with (
            tc.tile_pool(name="res", bufs=2 if reps > 1 else 1) as res,
            tc.tile_pool(name="win", bufs=5) as win,
            tc.tile_pool(name="ps1", bufs=7, space="PSUM") as ps1p,
            tc.tile_pool(name="ps2", bufs=1, space="PSUM") as ps2p,
        ):
            for rep in range(reps):
                xts = []
                for t in range(4):
                    xt = res.tile([128, NB], bf16, tag=f"xt{t}", name=f"xt{t}")
                    nc.sync.dma_start(xt[:], featT[t])
                    xts.append(xt)
                cws = []
                for t in range(4):
                    cw = res.tile([128, D], bf16, tag=f"cw{t}", name=f"cw{t}")
                    nc.sync.dma_start(cw[:], convw[t])
                    cws.append(cw)
                cbs = []
                for q in range(2):
                    cb = res.tile([128, 1], f32, tag=f"cb{q}", name=f"cb{q}")
                    nc.sync.dma_start(cb[:], convb[q])
                    cbs.append(cb)
                hts = [res.tile([128, NB], bf16, tag=f"ht{q}", name=f"ht{q}")
                       for q in range(2)]

                # Stage 1, k-outer so matmuls start as soon as xt[0] lands:
                # hT[q][:, n-tile] = relu(conv_w[:, q-half].T @ featT + b)
                for q in range(2):
                    pss = [ps1p.tile([128, NTILE], f32, tag="ps",
                                     name=f"ps{q}_{n}") for n in range(NTI)]
                    for k in range(4):
                        for n in range(NTI):
                            nc.tensor.matmul(
                                pss[n][:],
                                cws[k][:, q * 128:(q + 1) * 128],
                                xts[k][:, n * NTILE:(n + 1) * NTILE],
                                start=(k == 0),
                                stop=(k == 3),
                            )
                    for n in range(NTI):
                        nc.scalar.activation(
                            hts[q][:, n * NTILE:(n + 1) * NTILE],
                            pss[n][:],
                            mybir.ActivationFunctionType.Relu,
                            bias=cbs[q],
                        )

                # Stage 2: acc(16,192) += hT-slice(128,16).T @ W-tile(128,192)
                # q-major f-tile order; W stream chunked per CHUNKS.
                acc = ps2p.tile([16, 192], f32, tag="acc", name="acc")
                pos = 0
                for ch in CHUNKS:
                    wc = win.tile([128, ch * 192], bf16, tag="wc", name="wc")
                    nc.sync.dma_start(
                        wc[:], wmat[:, pos * 192:(pos + ch) * 192])
                    for t in range(ch):
                        p_ = pos + t
                        q, pix = (0, p_) if p_ < NQ else (1, p_ - NQ)
                        nc.tensor.matmul(
                            acc[:],
                            hts[q][:, pix * 16:(pix + 1) * 16],
                            wc[:, t * 192:(t + 1) * 192],
                            start=(p_ == 0),
                            stop=(p_ == NT - 1),
                        )
                    pos += ch

                ot = res.tile([16, 192], f32, tag="ot", name="ot")
                nc.vector.tensor_copy(ot[:], acc[:])
                nc.sync.dma_start(out[:] if reps == 1 else out[rep], ot[:])

    nc.compile()
    return nc


def _prep_inputs(features, conv_w, conv_b, cls_w, bbox_w):
    import ml_dtypes

    f32 = np.float32
    bf16 = ml_dtypes.bfloat16
    features = np.asarray(features, dtype=f32).astype(bf16)
    conv_w = np.asarray(conv_w, dtype=f32).astype(bf16)
    conv_b = np.ascontiguousarray(conv_b, dtype=f32)

    convw_t = np.ascontiguousarray(conv_w.reshape(4, 128, D))
    convb_t = conv_b.reshape(2, 128, 1)

    in_maps = []
    for i in range(NCORES):
        fi = features[:, i * HSH:(i + 1) * HSH, :, :].reshape(B, PIX, C)
        featT = np.ascontiguousarray(fi.transpose(2, 1, 0).reshape(C, NB))

        # wmat block t holds W rows for the t-th f-tile in q-major order:
        # t < NQ -> f-tile 2t (q=0), else f-tile 2(t-NQ)+1 (q=1).
        wl = np.empty((128, NT, 192), dtype=bf16)
        r0, r1 = i * FLOC, (i + 1) * FLOC
        cw3 = cls_w[r0:r1].astype(bf16).reshape(NT, 128, 64)
        bw3 = bbox_w[r0:r1].astype(bf16).reshape(NT, 128, 128)
        wl[:, :NQ, :64] = cw3[0::2].transpose(1, 0, 2)
        wl[:, NQ:, :64] = cw3[1::2].transpose(1, 0, 2)
        wl[:, :NQ, 64:] = bw3[0::2].transpose(1, 0, 2)
        wl[:, NQ:, 64:] = bw3[1::2].transpose(1, 0, 2)

        in_maps.append({
            "featT": featT.reshape(4, 128, NB),
            "convw": convw_t,
            "convb": convb_t,
            "wmat": wl.reshape(128, NT * 192),
        })
    return in_maps


def _run_device(in_maps, trace=False, **kw):
    from concourse.bass_utils import run_bass_kernel_spmd

    if "nc" not in _STATE:
        _STATE["nc"] = _build_module()
    nc = _STATE["nc"]
    return run_bass_kernel_spmd(
        nc, in_maps, core_ids=list(range(NCORES)), trace=trace, **kw
    )


def _postprocess(partial, roi, cls_b, bbox_b):
    f32 = np.float32
    cls = partial[:, :64].astype(f32) + cls_b.astype(f32)
    bbox = partial[:, 64:].astype(f32) + bbox_b.astype(f32)

    obj = 1.0 / (1.0 + np.exp(-(cls[:, P:] - cls[:, :P]), dtype=f32))
    bb = bbox.reshape(B, 4, P).transpose(0, 2, 1)
    roi_img = roi.astype(f32) * f32(STRIDE)
    x = roi_img[:, :, 0] - bb[:, :, 1] * roi_img[:, :, 3]
    y = roi_img[:, :, 1]
    w = roi_img[:, :, 2] * np.exp(np.clip(bb[:, :, 2], -10.0, 10.0), dtype=f32)
    hh = roi_img[:, :, 3] * np.exp(np.clip(bb[:, :, 3], -10.0, 10.0), dtype=f32)
    return np.stack([x, y, w, hh, obj], axis=-1).astype(f32)


def kernel(features, roi, conv_w, conv_b, cls_w, cls_b, bbox_w, bbox_b):
    in_maps = _prep_inputs(features, conv_w, conv_b, cls_w, bbox_w)
    res = _run_device(in_maps)
    partial = np.zeros((B, 192), dtype=np.float64)
    for r in res.results:
        partial += np.asarray(r["out"], dtype=np.float64)
    return _postprocess(partial.astype(np.float32), np.asarray(roi),
                        np.asarray(cls_b), np.asarray(bbox_b))



# revision 2
# speedup vs baseline: 1.8412x; 1.8412x over previous
"""Trainium2 Bass kernel for nn_ClassifierModel (nms_detection).

Computation (reference):
    h    = relu(features @ conv_w + conv_b)        # (B,H,W,C)@(C,D) -> (B,H,W,D)
    flat = h.reshape(B, F)                         # F = H*W*D = 401408
    cls  = flat @ cls_w + cls_b                    # (B, 64)
    bbox = flat @ bbox_w + bbox_b                  # (B, 128)
    <tiny postprocessing with roi -> (B, P, 5)>

Algebraic reductions exploited (host-side, exact):
  * obj = softmax([cls0, cls1])[1] = sigmoid(cls1 - cls0): only the 32-column
    weight difference cls_w[:,P:] - cls_w[:,:P] is needed, not all 64 columns.
  * The reference overwrites x, so bb[:,:,0] (bbox_w[:, :P]) is dead code:
    only 96 of the 128 bbox columns are needed.
  -> the streamed weight matrix is (F, 128) instead of (F, 192).

Sharding: the flatten (contraction) dim F is split across the 8 cores by
slicing H into 8 chunks of 28 rows. Each core computes its conv slice and a
partial (16, 128) product against its slice of the combined weights; the host
sums the 8 partials and runs the tiny postprocessing. This reads each
dense-weight element exactly once across the machine.

Precision/HBM-traffic strategy (the kernel is DMA-bound at 360 GB/s):
  * features are streamed as fp8 e3m4, pre-scaled by 2 on the host (undone
    for free via the activation `scale`); conv_w stays bf16.
  * h is written by the Relu activation directly as fp8 e4m3.
  * The combined dense weights are streamed as fp8 e4m3, pre-scaled by 1024
    on the host (their std ~0.001 would land in e4m3's subnormal range);
    the host divides the partials by 1024.
  * Stage-2 matmuls run in MatmulPerfMode.DoubleRow: each instruction
    contracts a PAIR of 128-row f-tiles (lhsT (128,2,16), rhs (128,2,128))
    at 0.5 cycles per output column - 4x the bf16 f-tile rate.

Per-core device layout:
    featT  (4,128,NB)      : features slice, (c, pix*B+b) columns, fp8 e3m4
    convw  (4,128,256)     : conv_w k-tiles (c on partitions), bf16
    convb  (2,128,1)       : conv_b halves (d on partitions), fp32
    wmat   (128,NPAIR*256) : combined W f-tile PAIRS in consumption order, e4m3
    out    (16,128)        : partial fp32 [cls-diff | bbox(P:)] sums

Stage 1 produces h^T with d on partitions and (pix, b) on the free axis; a
128-partition f-tile of flat^T is exactly hT[q][:, pix*16:(pix+1)*16], and a
DoubleRow pair is the contiguous 32-column slice hT[q][:, 32p:32p+32].
Stage 2 consumes pairs in q-major order so it can start as soon as hT[0]
exists; the host lays wmat out in the same order, with small tail chunks so
the final chunk's matmul tail is short.
"""

import numpy as np

B = 16
H, W, C = 224, 7, 512
D = 256
P = 32
NCORES = 8
HSH = H // NCORES          # 28 rows of H per core
PIX = HSH * W              # 196 pixels per core per batch
FLOC = PIX * D             # 50176 contraction elements per core
NB = PIX * B               # 3136 stage-1 moving columns
NT = FLOC // 128           # 392 f-tiles per core
NQ = NT // 2               # 196 f-tiles per d-half
NPAIR = NQ                 # 196 DoubleRow pairs (98 per d-half)
NPQ = NQ // 2              # 98 pairs per d-half
NTILE = 448                # stage-1 moving tile (3136 = 7*448)
NPT = NTILE // 16          # 28 pixels per stage-1 n-tile
KOUT = 128                 # streamed output columns: 32 cls-diff + 96 bbox
CHUNKS = [24] * 7 + [14, 8, 4, 2]   # W-stream chunks in pairs (sum = 196)
FEAT_SCALE = 2.0           # features pre-scale into e3m4 (undone in Relu)
W_SCALE = 1024.0           # dense-weight pre-scale into e4m3 (undone on host)
STRIDE = 16.0

_STATE = {}


def _build_module(reps=1):
    import concourse.mybir as mybir
    import concourse.tile as tile
    from concourse import bacc

    f32 = mybir.dt.float32
    bf16 = mybir.dt.bfloat16
    f8w = mybir.dt.float8e4
    f8f = mybir.dt.float8e3
    nc = bacc.Bacc("TRN2", target_bir_lowering=False, debug=False)

    featT = nc.dram_tensor("featT", [4, 128, NB], f8f, kind="ExternalInput")
    convw = nc.dram_tensor("convw", [4, 128, D], bf16, kind="ExternalInput")
    convb = nc.dram_tensor("convb", [2, 128, 1], f32, kind="ExternalInput")
    wmat = nc.dram_tensor("wmat", [128, NPAIR * 2 * KOUT], f8w,
                          kind="ExternalInput")
    if reps == 1:
        out = nc.dram_tensor("out", [16, KOUT], f32, kind="ExternalOutput")
    else:
        out = nc.dram_tensor("out", [reps, 16, KOUT], f32,
                             kind="ExternalOutput")

    NTI = NB // NTILE  # 7 stage-1 n-tiles

    with tile.TileContext(nc) as tc:
        with (
            tc.tile_pool(name="res", bufs=2 if reps > 1 else 1) as res,
            tc.tile_pool(name="win", bufs=5) as win,
            tc.tile_pool(name="ps1", bufs=7, space="PSUM") as ps1p,
            tc.tile_pool(name="ps2", bufs=1, space="PSUM") as ps2p,
        ):
            for rep in range(reps):
                xts = []
                for t in range(4):
                    xt = res.tile([128, NB], f8f, tag=f"xt{t}", name=f"xt{t}")
                    nc.sync.dma_start(xt[:], featT[t])
                    xts.append(xt)
                cws = []
                for t in range(4):
                    cw = res.tile([128, D], bf16, tag=f"cw{t}", name=f"cw{t}")
                    nc.sync.dma_start(cw[:], convw[t])
                    cws.append(cw)
                cbs = []
                for q in range(2):
                    cb = res.tile([128, 1], f32, tag=f"cb{q}", name=f"cb{q}")
                    nc.sync.dma_start(cb[:], convb[q])
                    cbs.append(cb)
                hts = [res.tile([128, NQ, 16], f8w, tag=f"ht{q}",
                                name=f"ht{q}") for q in range(2)]

                # Stage 1, k-outer so matmuls start as soon as xt[0] lands:
                # hT[q][:, n-tile] = relu((conv_w[:, q-half].T @ featT)/2 + b)
                for q in range(2):
                    pss = [ps1p.tile([128, NPT, 16], f32, tag="ps",
                                     name=f"ps{q}_{n}") for n in range(NTI)]
                    for k in range(4):
                        for n in range(NTI):
                            nc.tensor.matmul(
                                pss[n][:],
                                cws[k][:, q * 128:(q + 1) * 128],
                                xts[k][:, n * NTILE:(n + 1) * NTILE],
                                start=(k == 0),
                                stop=(k == 3),
                            )
                    for n in range(NTI):
                        nc.scalar.activation(
                            hts[q][:, n * NPT:(n + 1) * NPT, :],
                            pss[n][:],
                            mybir.ActivationFunctionType.Relu,
                            bias=cbs[q],
                            scale=1.0 / FEAT_SCALE,
                        )

                # Stage 2: acc(16,128) += pair-matmul in DoubleRow mode:
                # lhsT = hT[q][:, 2p:2p+2, :] (128,2,16), rhs = W pair
                # (128,2,128).  q-major pair order; W stream chunked.
                acc = ps2p.tile([16, KOUT], f32, tag="acc", name="acc")
                pos = 0
                for ch in CHUNKS:
                    wc = win.tile([128, ch, 2, KOUT], f8w, tag="wc", name="wc")
                    nc.sync.dma_start(
                        wc[:], wmat[:, pos * 2 * KOUT:(pos + ch) * 2 * KOUT])
                    for t in range(ch):
                        pp = pos + t
                        q, p = (0, pp) if pp < NPQ else (1, pp - NPQ)
                        nc.tensor.matmul(
                            acc[:],
                            hts[q][:, 2 * p:2 * p + 2, :],
                            wc[:, t],
                            start=(pp == 0),
                            stop=(pp == NPAIR - 1),
                            perf_mode=mybir.MatmulPerfMode.DoubleRow,
                        )
                    pos += ch

                ot = res.tile([16, KOUT], f32, tag="ot", name="ot")
                nc.vector.tensor_copy(ot[:], acc[:])
                nc.sync.dma_start(out[:] if reps == 1 else out[rep], ot[:])

    nc.compile()
    return nc


def _prep_inputs(features, conv_w, conv_b, cls_w, bbox_w):
    import ml_dtypes

    f32 = np.float32
    bf16 = ml_dtypes.bfloat16
    f8w = ml_dtypes.float8_e4m3
    f8f = ml_dtypes.float8_e3m4

    features = (np.asarray(features, dtype=f32) * f32(FEAT_SCALE)).astype(f8f)
    conv_w = np.asarray(conv_w, dtype=f32).astype(bf16)
    conv_b = np.ascontiguousarray(conv_b, dtype=f32)

    convw_t = np.ascontiguousarray(conv_w.reshape(4, 128, D))
    convb_t = conv_b.reshape(2, 128, 1)

    # Combined streamed weights: 32 cls-diff columns + 96 live bbox columns,
    # pre-scaled into e4m3's normal range.
    cls_w = np.asarray(cls_w, dtype=f32)
    bbox_w = np.asarray(bbox_w, dtype=f32)
    wcomb = np.empty((FLOC * NCORES, KOUT), dtype=f32)
    wcomb[:, :P] = cls_w[:, P:] - cls_w[:, :P]
    wcomb[:, P:] = bbox_w[:, P:]
    wcomb *= f32(W_SCALE)
    wcomb8 = wcomb.astype(f8w)

    in_maps = []
    for i in range(NCORES):
        fi = features[:, i * HSH:(i + 1) * HSH, :, :].reshape(B, PIX, C)
        featT = np.ascontiguousarray(fi.transpose(2, 1, 0).reshape(C, NB))

        # wmat pair pp (q-major: pp<98 -> q=0,p=pp; else q=1,p=pp-98) holds
        # the W rows of f-tiles (pix=2p, q) and (pix=2p+1, q):
        # wl[c, pp, i, j] = wcomb[r0 + (2p+i)*256 + q*128 + c, j]
        w4 = wcomb8[i * FLOC:(i + 1) * FLOC].reshape(PIX, 2, 128, KOUT)
        wl = np.empty((128, NPAIR, 2, KOUT), dtype=f8w)
        for q in range(2):
            pq = w4[:, q].reshape(NPQ, 2, 128, KOUT)  # (p, i, c, j)
            wl[:, q * NPQ:(q + 1) * NPQ] = pq.transpose(2, 0, 1, 3)

        in_maps.append({
            "featT": featT.reshape(4, 128, NB),
            "convw": convw_t,
            "convb": convb_t,
            "wmat": wl.reshape(128, NPAIR * 2 * KOUT),
        })
    return in_maps


def _run_device(in_maps, trace=False, **kw):
    from concourse.bass_utils import run_bass_kernel_spmd

    if "nc" not in _STATE:
        _STATE["nc"] = _build_module()
    nc = _STATE["nc"]
    return run_bass_kernel_spmd(
        nc, in_maps, core_ids=list(range(NCORES)), trace=trace, **kw
    )


def _postprocess(partial, roi, cls_b, bbox_b):
    f32 = np.float32
    partial = partial / f32(W_SCALE)
    cls_b = np.asarray(cls_b, dtype=f32)
    bbox_b = np.asarray(bbox_b, dtype=f32)

    diff = partial[:, :P] + (cls_b[P:] - cls_b[:P])
    obj = 1.0 / (1.0 + np.exp(-diff, dtype=f32))
    bb = (partial[:, P:] + bbox_b[P:]).reshape(B, 3, P)
    roi_img = np.asarray(roi, dtype=f32) * f32(STRIDE)
    x = roi_img[:, :, 0] - bb[:, 0, :] * roi_img[:, :, 3]
    y = roi_img[:, :, 1]
    w = roi_img[:, :, 2] * np.exp(np.clip(bb[:, 1, :], -10.0, 10.0), dtype=f32)
    hh = roi_img[:, :, 3] * np.exp(np.clip(bb[:, 2, :], -10.0, 10.0),
                                   dtype=f32)
    return np.stack([x, y, w, hh, obj], axis=-1).astype(f32)


def kernel(features, roi, conv_w, conv_b, cls_w, cls_b, bbox_w, bbox_b):
    in_maps = _prep_inputs(features, conv_w, conv_b, cls_w, bbox_w)
    res = _run_device(in_maps)
    partial = np.zeros((B, KOUT), dtype=np.float64)
    for r in res.results:
        partial += np.asarray(r["out"], dtype=np.float64)
    return _postprocess(partial.astype(np.float32), np.asarray(roi),
                        np.asarray(cls_b), np.asarray(bbox_b))


# revision 7
# speedup vs baseline: 2.2502x; 1.2221x over previous
"""Trainium2 Bass kernel for nn_ClassifierModel (nms_detection).

Computation (reference):
    h    = relu(features @ conv_w + conv_b)        # (B,H,W,C)@(C,D) -> (B,H,W,D)
    flat = h.reshape(B, F)                         # F = H*W*D = 401408
    cls  = flat @ cls_w + cls_b                    # (B, 64)
    bbox = flat @ bbox_w + bbox_b                  # (B, 128)
    <tiny postprocessing with roi -> (B, P, 5)>

Algebraic reductions exploited (host-side, exact):
  * obj = softmax([cls0, cls1])[1] = sigmoid(cls1 - cls0): only the 32-column
    weight difference cls_w[:,P:] - cls_w[:,:P] is needed, not all 64 columns.
  * The reference overwrites x, so bb[:,:,0] (bbox_w[:, :P]) is dead code:
    only 96 of the 128 bbox columns are needed.
  -> the streamed weight matrix is (F, 128) instead of (F, 192).

Sharding: the flatten (contraction) dim F is split across the 8 cores by
slicing H into 8 chunks of 28 rows. Each core computes its conv slice and a
partial (16, 128) product against its slice of the combined weights; the host
sums the 8 partials and runs the tiny postprocessing. This reads each
dense-weight element exactly once across the machine.

Precision/HBM-traffic strategy (the kernel is DMA-bound at 360 GB/s):
  * features are streamed as fp8 e3m4, pre-scaled by 2 on the host (undone
    for free via the activation `scale`); conv_w stays bf16.
  * h is written by the Relu activation directly as fp8 e4m3.
  * The combined dense weights are streamed as fp8 e4m3, pre-scaled by 1024
    on the host (their std ~0.001 would land in e4m3's subnormal range);
    the host divides the partials by 1024.
  * Stage-2 matmuls run in MatmulPerfMode.DoubleRow: each instruction
    contracts a PAIR of 128-row f-tiles (lhsT (128,2,16), rhs (128,2,128))
    at 0.5 cycles per output column - 4x the bf16 f-tile rate.

Per-core device layout:
    featT  (4,128,NB)      : features slice, (c, pix*B+b) columns, fp8 e3m4
    convw  (4,128,256)     : conv_w k-tiles (c on partitions), bf16
    convb  (2,128,1)       : conv_b halves (d on partitions), fp32
    wmat   (128,NPAIR*256) : combined W f-tile PAIRS in consumption order, e4m3
    out    (16,128)        : partial fp32 [cls-diff | bbox(P:)] sums

Stage 1 produces h^T with d on partitions and (pix, b) on the free axis; a
128-partition f-tile of flat^T is exactly hT[q][:, pix*16:(pix+1)*16], and a
DoubleRow pair is the contiguous 32-column slice hT[q][:, 32p:32p+32].
Stage 2 consumes pairs in q-major order so it can start as soon as hT[0]
exists; the host lays wmat out in the same order, with small tail chunks so
the final chunk's matmul tail is short.
"""

import numpy as np

B = 16
H, W, C = 224, 7, 512
D = 256
P = 32
NCORES = 8
HSH = H // NCORES          # 28 rows of H per core
PIX = HSH * W              # 196 pixels per core per batch
FLOC = PIX * D             # 50176 contraction elements per core
NB = PIX * B               # 3136 stage-1 moving columns
NT = FLOC // 128           # 392 f-tiles per core
NQ = NT // 2               # 196 f-tiles per d-half
NPAIR = NQ                 # 196 DoubleRow pairs (98 per d-half)
NPQ = NQ // 2              # 98 pairs per d-half
NTILE = 448                # stage-1 moving tile (3136 = 7*448)
NPT = NTILE // 16          # 28 pixels per stage-1 n-tile
KOUT = 128                 # streamed output columns: 32 cls-diff + 96 bbox
CHUNKS = [24] * 7 + [14, 8, 4, 2]   # W-stream chunks in pairs (sum = 196)
FEAT_SCALE = 2.0           # features pre-scale into e3m4 (undone in Relu)
W_SCALE = 1024.0           # dense-weight pre-scale into e4m3 (undone on host)
STRIDE = 16.0

_STATE = {}


def _build_module(reps=1):
    import concourse.mybir as mybir
    import concourse.tile as tile
    from concourse import bacc

    f32 = mybir.dt.float32
    bf16 = mybir.dt.bfloat16
    f8w = mybir.dt.float8e4
    f8f = mybir.dt.float8e3
    nc = bacc.Bacc("TRN2", target_bir_lowering=False, debug=False)

    featT = nc.dram_tensor("featT", [4, 128, NB], f8f, kind="ExternalInput")
    convw = nc.dram_tensor("convw", [4, 128, D], bf16, kind="ExternalInput")
    convb = nc.dram_tensor("convb", [2, 128, 1], f32, kind="ExternalInput")
    wmat = nc.dram_tensor("wmat", [128, NPAIR * 2 * KOUT], f8w,
                          kind="ExternalInput")
    if reps == 1:
        out = nc.dram_tensor("out", [16, KOUT], f32, kind="ExternalOutput")
    else:
        out = nc.dram_tensor("out", [reps, 16, KOUT], f32,
                             kind="ExternalOutput")

    NTI = NB // NTILE  # 7 stage-1 n-tiles
    NACT = 0           # leading W chunks issued from the Activation HWDGE

    with tile.TileContext(nc) as tc:
        with (
            tc.tile_pool(name="res", bufs=2 if reps > 1 else 1) as res,
            tc.tile_pool(name="win", bufs=1) as win,
            tc.tile_pool(name="ps1", bufs=7, space="PSUM") as ps1p,
            tc.tile_pool(name="ps2", bufs=1, space="PSUM") as ps2p,
        ):
            for rep in range(reps):
                # Small inputs first so no late-landing input DMA gates the
                # start of stage 1.
                cbs = []
                for q in range(2):
                    cb = res.tile([128, 1], f32, tag=f"cb{q}", name=f"cb{q}")
                    nc.sync.dma_start(cb[:], convb[q])
                    cbs.append(cb)
                xts, cws = [], []
                for t in range(4):
                    cw = res.tile([128, D], bf16, tag=f"cw{t}", name=f"cw{t}")
                    nc.sync.dma_start(cw[:], convw[t])
                    cws.append(cw)
                    xt = res.tile([128, NB], f8f, tag=f"xt{t}", name=f"xt{t}")
                    nc.sync.dma_start(xt[:], featT[t])
                    xts.append(xt)
                hts = [res.tile([128, NQ, 16], f8w, tag=f"ht{q}",
                                name=f"ht{q}") for q in range(2)]

                # The whole W stream is SBUF-resident (bufs=len(CHUNKS)), so
                # no chunk DMA ever waits on PE consumption. The first NACT
                # chunks issue from the Activation HWDGE queue so their
                # dispatch overlaps the SP queue's input DMAs; the rest go on
                # SP after the inputs.
                wcs = []
                pos = 0
                for ci, ch in enumerate(CHUNKS):
                    wc = win.tile([128, ch, 2, KOUT], f8w, tag=f"wc{ci}",
                                  name=f"wc{ci}")
                    eng = nc.scalar if ci < NACT else nc.sync
                    eng.dma_start(
                        wc[:], wmat[:, pos * 2 * KOUT:(pos + ch) * 2 * KOUT])
                    wcs.append((wc, pos, ch))
                    pos += ch

                # Stage 1, k-outer so matmuls start as soon as xt[0] lands:
                # hT[q][:, n-tile] = relu((conv_w[:, q-half].T @ featT)/2 + b)
                for q in range(2):
                    pss = [ps1p.tile([128, NPT, 16], f32, tag="ps",
                                     name=f"ps{q}_{n}") for n in range(NTI)]
                    for k in range(4):
                        for n in range(NTI):
                            nc.tensor.matmul(
                                pss[n][:],
                                cws[k][:, q * 128:(q + 1) * 128],
                                xts[k][:, n * NTILE:(n + 1) * NTILE],
                                start=(k == 0),
                                stop=(k == 3),
                            )
                    for n in range(NTI):
                        nc.scalar.activation(
                            hts[q][:, n * NPT:(n + 1) * NPT, :],
                            pss[n][:],
                            mybir.ActivationFunctionType.Relu,
                            bias=cbs[q],
                            scale=1.0 / FEAT_SCALE,
                        )

                # Stage 2: acc(16,128) += pair-matmul in DoubleRow mode:
                # lhsT = hT[q][:, 2p:2p+2, :] (128,2,16), rhs = W pair
                # (128,2,128).  q-major pair order matching the wmat layout.
                acc = ps2p.tile([16, KOUT], f32, tag="acc", name="acc")
                for wc, pos, ch in wcs:
                    for t in range(ch):
                        pp = pos + t
                        q, p = (0, pp) if pp < NPQ else (1, pp - NPQ)
                        nc.tensor.matmul(
                            acc[:],
                            hts[q][:, 2 * p:2 * p + 2, :],
                            wc[:, t],
                            start=(pp == 0),
                            stop=(pp == NPAIR - 1),
                            perf_mode=mybir.MatmulPerfMode.DoubleRow,
                        )

                ot = res.tile([16, KOUT], f32, tag="ot", name="ot")
                nc.vector.tensor_copy(ot[:], acc[:])
                nc.sync.dma_start(out[:] if reps == 1 else out[rep], ot[:])

    nc.compile()
    return nc


def _prep_inputs(features, conv_w, conv_b, cls_w, bbox_w):
    import ml_dtypes

    f32 = np.float32
    bf16 = ml_dtypes.bfloat16
    f8w = ml_dtypes.float8_e4m3
    f8f = ml_dtypes.float8_e3m4

    features = (np.asarray(features, dtype=f32) * f32(FEAT_SCALE)).astype(f8f)
    conv_w = np.asarray(conv_w, dtype=f32).astype(bf16)
    conv_b = np.ascontiguousarray(conv_b, dtype=f32)

    convw_t = np.ascontiguousarray(conv_w.reshape(4, 128, D))
    convb_t = conv_b.reshape(2, 128, 1)

    # Combined streamed weights: 32 cls-diff columns + 96 live bbox columns,
    # pre-scaled into e4m3's normal range.
    cls_w = np.asarray(cls_w, dtype=f32)
    bbox_w = np.asarray(bbox_w, dtype=f32)
    wcomb = np.empty((FLOC * NCORES, KOUT), dtype=f32)
    wcomb[:, :P] = cls_w[:, P:] - cls_w[:, :P]
    wcomb[:, P:] = bbox_w[:, P:]
    wcomb *= f32(W_SCALE)
    wcomb8 = wcomb.astype(f8w)

    in_maps = []
    for i in range(NCORES):
        fi = features[:, i * HSH:(i + 1) * HSH, :, :].reshape(B, PIX, C)
        featT = np.ascontiguousarray(fi.transpose(2, 1, 0).reshape(C, NB))

        # wmat pair pp (q-major: pp<98 -> q=0,p=pp; else q=1,p=pp-98) holds
        # the W rows of f-tiles (pix=2p, q) and (pix=2p+1, q):
        # wl[c, pp, i, j] = wcomb[r0 + (2p+i)*256 + q*128 + c, j]
        w4 = wcomb8[i * FLOC:(i + 1) * FLOC].reshape(PIX, 2, 128, KOUT)
        wl = np.empty((128, NPAIR, 2, KOUT), dtype=f8w)
        for q in range(2):
            pq = w4[:, q].reshape(NPQ, 2, 128, KOUT)  # (p, i, c, j)
            wl[:, q * NPQ:(q + 1) * NPQ] = pq.transpose(2, 0, 1, 3)

        in_maps.append({
            "featT": featT.reshape(4, 128, NB),
            "convw": convw_t,
            "convb": convb_t,
            "wmat": wl.reshape(128, NPAIR * 2 * KOUT),
        })
    return in_maps


def _run_device(in_maps, trace=False, **kw):
    from concourse.bass_utils import run_bass_kernel_spmd

    if "nc" not in _STATE:
        _STATE["nc"] = _build_module()
    nc = _STATE["nc"]
    return run_bass_kernel_spmd(
        nc, in_maps, core_ids=list(range(NCORES)), trace=trace, **kw
    )


def _postprocess(partial, roi, cls_b, bbox_b):
    f32 = np.float32
    partial = partial / f32(W_SCALE)
    cls_b = np.asarray(cls_b, dtype=f32)
    bbox_b = np.asarray(bbox_b, dtype=f32)

    diff = partial[:, :P] + (cls_b[P:] - cls_b[:P])
    obj = 1.0 / (1.0 + np.exp(-diff, dtype=f32))
    bb = (partial[:, P:] + bbox_b[P:]).reshape(B, 3, P)
    roi_img = np.asarray(roi, dtype=f32) * f32(STRIDE)
    x = roi_img[:, :, 0] - bb[:, 0, :] * roi_img[:, :, 3]
    y = roi_img[:, :, 1]
    w = roi_img[:, :, 2] * np.exp(np.clip(bb[:, 1, :], -10.0, 10.0), dtype=f32)
    hh = roi_img[:, :, 3] * np.exp(np.clip(bb[:, 2, :], -10.0, 10.0),
                                   dtype=f32)
    return np.stack([x, y, w, hh, obj], axis=-1).astype(f32)


def kernel(features, roi, conv_w, conv_b, cls_w, cls_b, bbox_w, bbox_b):
    in_maps = _prep_inputs(features, conv_w, conv_b, cls_w, bbox_w)
    res = _run_device(in_maps)
    partial = np.zeros((B, KOUT), dtype=np.float64)
    for r in res.results:
        partial += np.asarray(r["out"], dtype=np.float64)
    return _postprocess(partial.astype(np.float32), np.asarray(roi),
                        np.asarray(cls_b), np.asarray(bbox_b))
